# revision 47
# baseline (speedup 1.0000x reference)
"""AR LSTM decoder kernel for Trainium2, data-parallel over 8 NeuronCores.

Problem: per-step LSTM cell + FC(5) + log_softmax + argmax + class-embedding
feedback, B=1024, T=1024, IN=HIDDEN=64. Batch is sharded 128 rows/core; the
T=1024 recurrence runs on-device per core. All recurrent state is kept in
[feature, batch] layout so no per-step transposes are needed; the only
transpose is the [B,5] argmax one-hot -> [5,B], done on the PE.

log_softmax is applied on host (argmax(logits) == argmax(log_softmax(logits)),
so the device only needs biased logits for both output and feedback).
"""
import os
import sys

sys.path.insert(0, "/opt/trn_rl_repo")

import numpy as np

import concourse.bass as bass
import concourse.tile as tile
from concourse.tile import add_dep_helper
from concourse import mybir
from concourse.bass_utils import run_bass_kernel_spmd

P = 128          # batch rows per core
IN = 64
H = 64
NCLS = 5
NCORES = 8
XB = 64          # time steps per x-block DMA
S_OUT = 128      # time steps per output accumulation chunk

F32 = mybir.dt.float32


def _split_excess_waits(nc, cap=1):
    """This walrus build accepts at most one sync-wait per instruction; move
    excess waits onto preceding same-engine NOPs (equivalent ordering)."""
    n_new = 0
    for f in nc.m.functions:
        for bb in f.blocks:
            new_list = None
            for idx, inst in enumerate(bb.instructions):
                si = inst.sync_info
                waits = list(si.on_wait) if si and si.on_wait else []
                if len(waits) > cap:
                    if new_list is None:
                        new_list = list(bb.instructions[:idx])
                    extra, keep = waits[:-cap], waits[-cap:]
                    for w in extra:
                        n_new += 1
                        new_list.append(mybir.InstNoOp(
                            name=f"waitsplit-{n_new}-{inst.name}",
                            sync_info=mybir.SyncInfo(on_wait=[w], on_update=[]),
                            bass_nofuse=True,
                            engine=inst.engine,
                        ))
                    inst.sync_info = mybir.SyncInfo(
                        on_wait=keep, on_update=list(si.on_update or []))
                    new_list.append(inst)
                elif new_list is not None:
                    new_list.append(inst)
            if new_list is not None:
                bb.instructions = new_list
    return n_new


def _build(t_steps, groups=1):
    """The kernel is latency-bound: per-step time equals one dependency-chain
    traversal, so the build minimizes chain units (groups=1; batch-splitting
    into pipelined groups was measured slower - each group still pays the full
    chain per step and the halves contend for engines). All per-step PSUM
    tensors share a single bank (matmul outputs write disjoint column ranges;
    only the first matmul of a bank instance uses start=True)."""
    nc = bass.Bass("TRN2", target_bir_lowering=False)

    xT = nc.dram_tensor("xT", [t_steps, IN, P], F32, kind="ExternalInput")[:]
    y = nc.dram_tensor("y", [P, t_steps * NCLS], F32, kind="ExternalOutput")[:]
    # late lhsT = [W_hh_gate.T ; bias ; zero pad ; (emb @ W_prev_gate).T]
    # ohT lives at partition 96 (engine accesses need 32-aligned bases);
    # rows 65:96 are zero weights against dead state rows
    KL = 101
    w_if_x = nc.dram_tensor("w_if_x", [IN, 128], F32, kind="ExternalInput")[:]
    w_if_l = nc.dram_tensor("w_if_l", [KL, 128], F32, kind="ExternalInput")[:]
    w_go_x = nc.dram_tensor("w_go_x", [IN, 128], F32, kind="ExternalInput")[:]
    w_go_l = nc.dram_tensor("w_go_l", [KL, 128], F32, kind="ExternalInput")[:]
    wfcb = nc.dram_tensor("wfcb", [H + 1, NCLS], F32, kind="ExternalInput")[:]
    ident = nc.dram_tensor("ident", [P, P], F32, kind="ExternalInput")[:]

    sig = mybir.ActivationFunctionType.Sigmoid
    tanh = mybir.ActivationFunctionType.Tanh

    GB = P // groups
    with tile.TileContext(nc) as tc:
        with (
            tc.tile_pool(name="const", bufs=1) as const,
            tc.tile_pool(name="state", bufs=1) as state,
            tc.tile_pool(name="xblk", bufs=3) as xblk,
            tc.tile_pool(name="work", bufs=3) as work,
            tc.tile_pool(name="acc", bufs=2) as accp,
            tc.tile_pool(name="psA", bufs=2, space="PSUM") as psA,
            tc.tile_pool(name="psB", bufs=2, space="PSUM") as psB,
        ):
            # constants
            c_wifx = const.tile([IN, 128], F32, tag="wifx")
            c_wifl = const.tile([KL, 128], F32, tag="wifl")
            c_wgox = const.tile([IN, 128], F32, tag="wgox")
            c_wgol = const.tile([KL, 128], F32, tag="wgol")
            c_wfcb = const.tile([H + 1, NCLS], F32, tag="wfcb")
            c_id = const.tile([P, P], F32, tag="ident")
            for dst, src in ((c_wifx, w_if_x), (c_wifl, w_if_l),
                             (c_wgox, w_go_x), (c_wgol, w_go_l),
                             (c_wfcb, wfcb), (c_id, ident)):
                nc.sync.dma_start(out=dst[:], in_=src)

            # per-group persistent state, concatenated so the late gate matmul
            # is a single contraction: rows 0:64 h, row 64 ones (bias), rows
            # 96:101 onehot^T (embedding feedback; 65:96 dead zero-weight pad)
            scats, cs = [], []
            for g in range(groups):
                scat = state.tile([KL, GB], F32, tag=f"scat{g}")
                # c lives at base partition 64 so the f-half of sigma(i,f)
                # (partitions 64:128) can multiply it SBUF-to-SBUF
                c_t = state.tile([128, GB], F32, tag=f"c{g}")
                nc.vector.memset(scat[:], 0.0)
                nc.vector.memset(scat[64:65, :], 1.0)
                nc.vector.memset(c_t[:], 0.0)
                scats.append(scat); cs.append(c_t)

            acc = None
            xb = None

            def part1(t, g, x_t):
                """gates matmuls -> activations -> cell -> h -> fc logits"""
                scat, c = scats[g], cs[g]
                gb0 = g * GB
                # one PSUM bank holds all PE outputs of this (group, step):
                # if 0:GB | go GB:2GB | log | ohT transposed
                bankA = psA.tile([128, 512], F32, tag=f"bankA{g}")
                ps_if = bankA[:, 0:GB]
                ps_go = bankA[:, GB:2 * GB]
                ps_log = bankA[gb0:gb0 + GB, 2 * GB:2 * GB + NCLS]

                # x contributions first (prefetchable, off the critical chain)
                mm1 = nc.tensor.matmul(ps_if, c_wifx[:], x_t, start=True,
                                       stop=False, skip_group_check=True)
                mm_gx = nc.tensor.matmul(ps_go, c_wgox[:], x_t, start=False,
                                         stop=False, skip_group_check=True)
                # late contributions: [h; ones; ohT] against fused weights
                mm_il = nc.tensor.matmul(ps_if, c_wifl[:], scat[:], start=False,
                                         stop=True, skip_group_check=True)
                mm_gl = nc.tensor.matmul(ps_go, c_wgol[:], scat[:], start=False,
                                         stop=True, skip_group_check=True)
                # only mm1 clears the bank's has_written bits; every other PE
                # write of this bank instance must execute after it
                mms = [mm_gx, mm_il, mm_gl]

                # SBUF-to-SBUF cell math: all pair bases align at 64 (f-half of
                # sig_if with c) or are partition-shifted (proven legal on HW)
                c_t = c
                sig_if = work.tile([128, GB], F32, tag=f"sigif{g}")
                nc.scalar.activation(sig_if[:], ps_if, sig)
                tg = work.tile([H, GB], F32, tag=f"tg{g}")
                nc.scalar.activation(tg[:], ps_go[0:64, :], tanh)
                so = work.tile([H, GB], F32, tag=f"so{g}")
                nc.scalar.activation(so[:], ps_go[64:128, :], sig)

                m1t = work.tile([128, GB], F32, tag=f"m1{g}")
                nc.vector.tensor_mul(m1t[64:128, :], sig_if[64:128, :], c_t[64:128, :])
                m2t = work.tile([128, GB], F32, tag=f"m2{g}")
                # output partition-shifted to base 64 so the add is base-aligned
                nc.vector.tensor_mul(m2t[64:128, :], sig_if[0:64, :], tg[:])
                nc.vector.tensor_add(c_t[64:128, :], m1t[64:128, :], m2t[64:128, :])
                tc_ = work.tile([H, GB], F32, tag=f"tc{g}")
                nc.scalar.activation(tc_[:], c_t[64:128, :], tanh)
                nc.vector.tensor_mul(scat[0:64, :], so[:], tc_[:])

                # fc with bias folded via the ones row; output partitions at
                # gb0 so the acc copy keeps input/output lanes aligned
                mmf = nc.tensor.matmul(ps_log, scat[0:H + 1, :], c_wfcb[:],
                                       start=False, stop=True,
                                       skip_group_check=True)
                mms.append(mmf)
                for m in mms:
                    add_dep_helper(m.ins, mm1.ins, sync=False,
                                   reason="bank start order")
                return bankA, mm1

            def part2(t, g, bank_mm, acc, s5):
                """argmax -> onehot -> transpose -> ohT; output copy"""
                scat = scats[g]
                gb0 = g * GB
                bankA, mm1 = bank_mm
                ps_log = bankA[gb0:gb0 + GB, 2 * GB:2 * GB + NCLS]
                ps_oh = bankA[0:NCLS, 2 * GB + 8:2 * GB + 8 + GB]
                mx = work.tile([128, 1], F32, tag=f"mx{g}")
                mxs = mx[gb0:gb0 + GB, :]
                nc.vector.reduce_max(mxs, ps_log, axis=mybir.AxisListType.X)
                onehot = work.tile([128, NCLS], F32, tag=f"oh{g}")
                ohs = onehot[gb0:gb0 + GB, :]
                nc.vector.tensor_scalar(ohs, ps_log, mxs, None,
                                        op0=mybir.AluOpType.is_ge)
                mmt = nc.tensor.transpose(
                    ps_oh, ohs, c_id[gb0:gb0 + GB, gb0:gb0 + GB])
                add_dep_helper(mmt.ins, mm1.ins, sync=False, reason="bank order")
                nc.vector.tensor_copy(scat[96:96 + NCLS, :], ps_oh)
                # output copy on ACT: off the critical chain, and keeps the
                # in-order DVE queue free for the next step's chain ops
                nc.scalar.copy(acc[gb0:gb0 + GB, s5:s5 + NCLS], ps_log)

            # Emission order staggers the groups half a step so the in-order
            # engine queues interleave ready work from both chains:
            #   part1(A,t), part2(B,t-1), part1(B,t), part2(A,t)
            pend = {g: None for g in range(groups)}   # g -> (bank, mm1, acc, s5)
            accs = {}
            for t in range(t_steps):
                sb, so_in_blk = divmod(t, XB)
                if so_in_blk == 0:
                    nsteps = min(XB, t_steps - sb * XB)
                    xb = xblk.tile([IN, XB, P], F32, tag="xb")
                    nc.sync.dma_start(
                        out=xb[:, :nsteps, :],
                        in_=xT[sb * XB: sb * XB + nsteps].rearrange("t f b -> f t b"),
                    )
                if t % S_OUT == 0:
                    acc = accp.tile([P, S_OUT * NCLS], F32, tag="acc")
                s5 = (t % S_OUT) * NCLS

                for g in range(groups):
                    x_t = xb[:, so_in_blk, g * GB:(g + 1) * GB]
                    bm = part1(t, g, x_t)
                    if groups == 1:
                        part2(t, g, bm, acc, s5)
                    else:
                        other = 1 - g
                        if pend[other] is not None:
                            part2(*pend[other])
                            pend[other] = None
                        pend[g] = (t, g, bm, acc, s5)

                if (t + 1) % S_OUT == 0 or t == t_steps - 1:
                    for g in range(groups):
                        if pend[g] is not None:
                            part2(*pend[g])
                            pend[g] = None
                    t0 = (t // S_OUT) * S_OUT
                    ncols = (t - t0 + 1) * NCLS
                    nc.sync.dma_start(
                        out=y[:, t0 * NCLS: t0 * NCLS + ncols],
                        in_=acc[:, :ncols],
                    )

    _split_excess_waits(nc, cap=1)
    return nc


_BUILT = {}


def _get_nc(t_steps):
    if t_steps not in _BUILT:
        _BUILT[t_steps] = _build(t_steps)
    return _BUILT[t_steps]


def _prep_maps(x, W_ih, W_hh, b_ih, b_hh, W_fc, b_fc, emb, t_steps):
    x = np.asarray(x, np.float32)
    W_ih = np.asarray(W_ih, np.float32)
    W_hh = np.asarray(W_hh, np.float32)
    b = (np.asarray(b_ih, np.float32) + np.asarray(b_hh, np.float32))
    W_fc = np.asarray(W_fc, np.float32)
    b_fc = np.asarray(b_fc, np.float32)
    emb = np.asarray(emb, np.float32)

    com = {
        "w_if_x": np.ascontiguousarray(W_ih[0:128, 0:64].T),
        "w_if_l": np.ascontiguousarray(np.vstack([
            W_hh[0:128].T, b[0:128][None, :], np.zeros((31, 128), np.float32),
            emb @ W_ih[0:128, 64:128].T])),
        "w_go_x": np.ascontiguousarray(W_ih[128:256, 0:64].T),
        "w_go_l": np.ascontiguousarray(np.vstack([
            W_hh[128:256].T, b[128:256][None, :], np.zeros((31, 128), np.float32),
            emb @ W_ih[128:256, 64:128].T])),
        "wfcb": np.ascontiguousarray(np.vstack([W_fc.T, b_fc[None, :]])),
        "ident": np.eye(P, dtype=np.float32),
    }
    xt = np.ascontiguousarray(x.transpose(1, 2, 0))  # [T, 64, B_full]
    maps = []
    for cidx in range(NCORES):
        m = dict(com)
        m["xT"] = np.ascontiguousarray(xt[:, :, cidx * P:(cidx + 1) * P])
        maps.append(m)
    return maps


def kernel(x, x_lengths=None, edge_list=None, W_ih=None, W_hh=None,
           b_ih=None, b_hh=None, W_fc=None, b_fc=None, emb=None, **_):
    x = np.asarray(x, np.float32)
    B, t_steps, _ = x.shape
    assert B == P * NCORES
    nc = _get_nc(t_steps)
    maps = _prep_maps(x, W_ih, W_hh, b_ih, b_hh, W_fc, b_fc, emb, t_steps)
    res = run_bass_kernel_spmd(nc, maps, core_ids=list(range(NCORES)))
    shards = [res.results[i]["y"].reshape(P, t_steps, NCLS) for i in range(NCORES)]
    logits = np.concatenate(shards, axis=0)
    m = logits.max(-1, keepdims=True)
    logp = logits - m - np.log(np.exp(logits - m).sum(-1, keepdims=True))
    return logp.astype(np.float32)
